# revision 28
# baseline (speedup 1.0000x reference)
"""Trainium2 Bass kernel for nn_AvgPoolVectorsPerWSI (segment-mean over groups).

Math: x [N=2048, M=512, 7, 7], idx [N] in [0,64)
  out[g, m] = mean over {n: idx[n]==g} and spatial of x[n, m, :, :]  -> [64, 512, 1, 1]

Strategy (no collectives needed):
  - Shard over M: core k handles an m-slice of 64 channels. Each core reads
    its x slice [2048, 64, 49] (25.7 MB) once; the stream runs at the
    per-core HBM cap (~353-390 GB/s depending on device state), which is
    the roofline. Tiles 0..13 are single full DMAs: small stripes push the
    straggler SDMA engine below the cap.
  - Work split sized so BOTH engines keep the DMA pace even on fast-HBM
    days (~4.0 us/tile): TensorE takes ch [0:20] raw (two 490-col fp32
    matmuls/tile, segment-sum fused on raw columns, ~2.8 us); VectorE
    j-reduces ch [20:64] (~2.9 us) into xs and TensorE adds one small
    [128,44] matmul into psum_small.
  - PSUM: the two raw chunks write ONE bank partition-stacked
    (psum_big[0:64] = ch 0:10, [64:128] = ch 10:20), so the final j-reduce
    of the accumulated bank is a single 128-lane tensor_reduce (0.7 us
    instead of 2 us at 64 lanes). start=True only on each chunk's first
    write (tile 0): a later start would clear the bank's has_written bits
    and drop the other half's accumulation (probed).
  - Tail: tiles 14-15 stream DVE-part first ([d | c] pieces; tile 15 as
    [d1 | c0 | c1 | d2 | d3] with shrinking d-pieces) so both engines
    chase the last bytes. After the last piece: small reduce -> small
    matmul -> SUBRED/copy -> three small output DMAs split across the ACT
    HWDGE ring and Sync, leaving only ~2-3 us + DMA latency after the
    stream.

Raw Block implementation (not Tile): the walrus matmul/DMA lowerings only
accept ONE attached sync-wait per instruction; standalone wait_ge
instructions sidestep that.

DMA-completion semaphores: one sem per in-flight piece (slot-cycled, with
cumulative thresholds). Distinct pieces may NOT share a sem with
intermediate thresholds: SDMA engines progress unevenly (engine 15 runs
~15-20% slow), so a later piece's stripes could satisfy an earlier piece's
count while that piece is still in flight. Per-piece sems + FIFO-per-engine
ordering make each threshold exact.
"""

from contextlib import ExitStack

import numpy as np

import concourse.bass as bass
import concourse.mybir as mybir
from concourse.bass_utils import run_bass_kernel_spmd

N = 2048          # samples
M = 512           # channels
HW = 49           # spatial (7*7)
G = 64            # groups
CORES = 8
ML = M // CORES   # 64 channels per core
F = ML * HW       # 3136 floats per (n, core)
P = 128           # partitions per tile
NT = N // P       # 16 n-tiles
BUFS = 8          # x-tile buffer depth (slots)

CPE = 18          # channels via the raw TensorE path (2 x 441-col chunks)
FC = CPE * HW     # 980 raw columns
HC = FC // 2      # 490 columns per chunk (one per psum partition-half)
MV = ML - CPE     # 44 channels via the VectorE reduce path

# Per-tile DMA pieces as column ranges, in stream order. d = DVE columns
# [FC:F], c = PE columns. Tiles 14-15 stream d-parts first so VectorE can
# chase; tile 15's d splits into shrinking pieces.
PIECES = {t: [(0, F)] for t in range(NT)}
PIECES[NT - 2] = [(FC, F), (0, FC)]
PIECES[NT - 1] = [
    (FC, FC + 24 * HW),            # d1: ch [18:42]
    (0, HC),                       # c0
    (HC, FC),                      # c1
    (FC + 24 * HW, FC + 38 * HW),  # d2: ch [42:56]
    (FC + 38 * HW, FC + 44 * HW),  # d3: ch [56:62]
    (FC + 44 * HW, F),             # d4: ch [62:64] — tiny last piece so
]                                  # the final reduce+matmul chain is short
# (piece index holding d / c data per tile)
D_PIECES = {t: [(0, (FC, F))] for t in range(NT - 1)}
D_PIECES[NT - 1] = [(0, PIECES[NT - 1][0]), (3, PIECES[NT - 1][3]),
                    (4, PIECES[NT - 1][4]), (5, PIECES[NT - 1][5])]
C_PIECE = {t: 0 for t in range(NT - 1)}
C_PIECE[NT - 2] = 1

NRED = NT - 1 + len(D_PIECES[NT - 1])   # total d-reduce ops (15 + 3)
# matmuls: c0, c1, small per tile 0..14; c0, c1, 3 small blocks for t15
NPE = 3 * (NT - 1) + 2 + len(D_PIECES[NT - 1])

F32 = mybir.dt.float32


def _build():
    nc = bass.Bass(trn_type="TRN2", target_bir_lowering=False)
    x_ext = nc.declare_dram_parameter("x", [N, F], F32, isOutput=False)
    # aux[:, 0:64] iota row, aux[:, 64:128] scale row, aux[:, 128:144] idx
    aux_ext = nc.declare_dram_parameter("aux", [P, G + G + NT], F32,
                                        isOutput=False)
    out_ext = nc.declare_dram_parameter("out", [G, ML], F32, isOutput=True)

    x_t = x_ext.ap().rearrange("(t p) f -> t p f", p=P)  # [16, 128, 3136]

    with ExitStack() as ctx:
        x_buf = ctx.enter_context(nc.sbuf_tensor([P, BUFS * F], F32))
        xs_buf = ctx.enter_context(nc.sbuf_tensor([P, BUFS * MV], F32))
        aux_sb = ctx.enter_context(nc.sbuf_tensor([P, G + G + NT], F32))
        w_sb = ctx.enter_context(nc.sbuf_tensor([P, NT * G], F32))
        out_sb = ctx.enter_context(nc.sbuf_tensor([G, MV], F32))
        scratch_sb = ctx.enter_context(nc.sbuf_tensor([1, 1], F32))
        tmp_a = ctx.enter_context(nc.sbuf_tensor([G, CPE // 2], F32))
        tmp_b = ctx.enter_context(nc.sbuf_tensor([G, CPE // 2], F32))
        # two separate banks: 64-row matmuls into the SAME bank's two
        # partition halves run on both PE column-groups concurrently and
        # halve the rhs stream rate (measured 2.2x slower passes)
        psum_bigA = ctx.enter_context(nc.psum_tensor([G, HC], F32))
        psum_bigB = ctx.enter_context(nc.psum_tensor([G, HC], F32))
        psum_small = ctx.enter_context(nc.psum_tensor([G, MV], F32))
        dma_x = [
            [
                ctx.enter_context(nc.semaphore(name=f"dma_x{s}_{p}"))
                for p in range(max(len(PIECES[t]) for t in range(s, NT, BUFS)))
            ]
            for s in range(BUFS)
        ]
        dma_a = ctx.enter_context(nc.semaphore())   # +16 when aux resident
        dma_o = ctx.enter_context(nc.semaphore())   # +16 per out DMA (3)
        wg_sem = ctx.enter_context(nc.semaphore())  # +1 when w generated
        red_sem = ctx.enter_context(nc.semaphore())  # +1 per d-reduce
        pe_sem = ctx.enter_context(nc.semaphore())   # +1 per matmul
        cp_sem = ctx.enter_context(nc.semaphore())   # +1 when out_sb copied
        sub_sem = ctx.enter_context(nc.semaphore())  # +1 when SUBRED done
        block = ctx.enter_context(nc.Block())

        def xsem(t, p):
            # piece p of tile t: its sem and cumulative threshold
            slot = t % BUFS
            uses = sum(
                1 for tp in range(slot, t + 1, BUFS) if len(PIECES[tp]) > p
            )
            return dma_x[slot][p], 16 * uses

        # ---- DMA program (SP / HWDGE, FIFO): the x stream ----
        @block.sync
        def _(sync):
            def xdma(t):
                if t >= BUFS:
                    # slot reuse: x_buf slot free once the previous tile's
                    # d-columns are reduced AND its raw chunks are matmul'd
                    sync.wait_ge(red_sem, t - BUFS + 1)
                    sync.wait_ge(pe_sem, 3 * (t - BUFS) + 2)
                slot = t % BUFS
                for p, (lo, hi) in enumerate(PIECES[t]):
                    sem, thr = xsem(t, p)
                    sync.dma_start(
                        out=x_buf[:, slot * F + lo:slot * F + hi],
                        in_=x_t[t][:, lo:hi],
                    ).then_inc(sem, 16)

            xdma(0)
            sync.dma_start(out=aux_sb[:, :], in_=aux_ext.ap()).then_inc(dma_a, 16)
            for t in range(1, NT):
                xdma(t)
            # out DMAs for the raw-path channels as each SUBRED lands
            sync.wait_ge(sub_sem, 1)
            sync.dma_start(
                out=out_ext.ap()[:, 0:CPE // 2], in_=tmp_a[:, :]
            ).then_inc(dma_o, 16)
            sync.wait_ge(sub_sem, 2)
            sync.dma_start(
                out=out_ext.ap()[:, CPE // 2:CPE], in_=tmp_b[:, :]
            ).then_inc(dma_o, 16)
            sync.wait_ge(dma_o, 48)

        # ---- VectorE: w generation, d-reduces, final stacked SUBRED ----
        @block.vector
        def _(vector):
            # w[p, t*G+g] = (idx[t*128+p] == g) * scale[g]
            vector.wait_ge(dma_a, 16)
            for t in range(NT):
                wg = vector.scalar_tensor_tensor(
                    out=w_sb[:, t * G:(t + 1) * G],
                    in0=aux_sb[:, 0:G],
                    scalar=aux_sb[:, 2 * G + t:2 * G + t + 1],
                    in1=aux_sb[:, G:2 * G],
                    op0=mybir.AluOpType.is_equal,
                    op1=mybir.AluOpType.mult,
                )
            wg.then_inc(wg_sem, 1)

            def dred(t, p, lo, hi):
                slot = t % BUFS
                sem, thr = xsem(t, p)
                vector.wait_ge(sem, thr)
                vector.tensor_reduce(
                    out=xs_buf[
                        :,
                        slot * MV + (lo - FC) // HW:
                        slot * MV + (hi - FC) // HW,
                    ],
                    in_=x_buf[
                        :, slot * F + lo:slot * F + hi
                    ].rearrange("p (m j) -> p m j", j=HW),
                    axis=mybir.AxisListType.X,
                    op=mybir.AluOpType.add,
                ).then_inc(red_sem, 1)

            def subred(tmp, psum, need):
                vector.wait_ge(pe_sem, need)
                vector.tensor_reduce(
                    out=tmp[:, :],
                    in_=psum[:, :].rearrange("p (m j) -> p m j", j=HW),
                    axis=mybir.AxisListType.X,
                    op=mybir.AluOpType.add,
                ).then_inc(sub_sem, 1)

            for t in range(NT - 1):
                if t >= BUFS:
                    # xs slot reuse: tile t-BUFS consumed by its small mm
                    vector.wait_ge(pe_sem, 3 * (t - BUFS) + 3)
                ((p, (lo, hi)),) = D_PIECES[t]
                dred(t, p, lo, hi)
            # tile 15: chase the shrinking d-pieces, interleaving the two
            # psum_big j-reduces once the raw chunks' matmuls retire
            t = NT - 1
            vector.wait_ge(pe_sem, 3 * (t - BUFS) + 3)
            dred(t, *D_PIECES[t][0][0:1], *D_PIECES[t][0][1])
            dred(t, *D_PIECES[t][1][0:1], *D_PIECES[t][1][1])
            subred(tmp_a, psum_bigA, 3 * (NT - 1) + 1)   # t15 c0 matmul done
            dred(t, *D_PIECES[t][2][0:1], *D_PIECES[t][2][1])
            dred(t, *D_PIECES[t][3][0:1], *D_PIECES[t][3][1])
            subred(tmp_b, psum_bigB, 3 * (NT - 1) + 2)   # t15 c1 matmul done

        # ---- TensorE: raw chunk matmuls + small reduced matmul ----
        @block.tensor
        def _(tensor):
            tensor.wait_ge(wg_sem, 1)
            for t in range(NT):
                slot = t % BUFS
                wt = w_sb[:, t * G:(t + 1) * G]
                last = t == NT - 1
                if last:
                    csems = [xsem(t, 1), xsem(t, 2)]
                else:
                    csems = [xsem(t, C_PIECE[t])] * 2
                for half, (sem, thr) in enumerate(csems):
                    if half == 0 or last:
                        tensor.wait_ge(sem, thr)
                    tensor.matmul(
                        out=(psum_bigA if half == 0 else psum_bigB)[:, :],
                        lhsT=wt,
                        rhs=x_buf[
                            :, slot * F + half * HC:slot * F + (half + 1) * HC
                        ],
                        start=(t == 0),
                        stop=last,
                    ).then_inc(pe_sem, 1)
                if not last:
                    tensor.wait_ge(red_sem, t + 1)
                    tensor.matmul(
                        out=psum_small[:, :],
                        lhsT=wt,
                        rhs=xs_buf[:, slot * MV:(slot + 1) * MV],
                        start=(t == 0),
                        stop=False,
                    ).then_inc(pe_sem, 1)
                else:
                    # chase the d-piece reduces with three column-block mms
                    blocks = [
                        ((lo - FC) // HW, (hi - FC) // HW)
                        for _, (lo, hi) in D_PIECES[t]
                    ]
                    for k, (blo, bhi) in enumerate(blocks):
                        tensor.wait_ge(red_sem, NT - 1 + k + 1)
                        tensor.matmul(
                            out=psum_small[:, blo:bhi],
                            lhsT=wt,
                            rhs=xs_buf[
                                :, slot * MV + blo:slot * MV + bhi
                            ],
                            start=False,
                            stop=True,
                        ).then_inc(pe_sem, 1)

        # ---- ACT (scalar): psum_small copy + its out DMA (2nd HWDGE ring)
        @block.scalar
        def _(scalar):
            # dummy copy at kernel start: pulls the lazy ACT_TABLE_LOAD
            # (~1.5 us) off the critical tail chain
            scalar.copy(scratch_sb[:, :], aux_sb[0:1, 0:1])
            scalar.wait_ge(pe_sem, NPE)
            scalar.copy(out_sb[:, :], psum_small[:, :]).then_inc(cp_sem, 1)
            # engines pipeline dispatch: gate the DMA on the copy's landing
            scalar.wait_ge(cp_sem, 1)
            scalar.dma_start(
                out=out_ext.ap()[:, CPE:ML], in_=out_sb[:, :]
            ).then_inc(dma_o, 16)

    return nc


def _prepare(x, idx):
    x = np.asarray(x)
    if x.dtype != np.float32:
        x = x.astype(np.float32)
    idx = np.asarray(idx).astype(np.int64)
    counts = np.bincount(idx, minlength=G).astype(np.float64)
    scale = np.where(counts > 0, 1.0 / (counts * HW), 0.0).astype(np.float32)
    aux = np.zeros((P, G + G + NT), np.float32)
    aux[:, 0:G] = np.arange(G, dtype=np.float32)[None, :]
    aux[:, G:2 * G] = scale[None, :]
    aux[:, 2 * G:] = idx.reshape(NT, P).T.astype(np.float32)
    xr = x.reshape(N, M, HW)
    in_maps = []
    for k in range(CORES):
        shard = np.ascontiguousarray(xr[:, k * ML:(k + 1) * ML, :]).reshape(N, F)
        in_maps.append({"x": shard, "aux": aux})
    return in_maps


def run(x, tensor_list_assignmentindices, trace=False):
    in_maps = _prepare(x, tensor_list_assignmentindices)
    nc = _build()
    res = run_bass_kernel_spmd(nc, in_maps, core_ids=list(range(CORES)), trace=trace)
    outs = [np.asarray(r["out"]) for r in res.results]
    out = np.concatenate(outs, axis=1)  # [G, M]
    return out.reshape(G, M, 1, 1).astype(np.float32), res.exec_time_ns


def kernel(**inputs):
    out, _ = run(inputs["x"], inputs["tensor_list_assignmentindices"], trace=False)
    return out
